# revision 39
# baseline (speedup 1.0000x reference)
"""Trainium2 Bass kernel for nn_EnhancedDDGAttention.

Sharding: data-parallel over the batch axis N=8 -> one batch element per
NeuronCore (8 cores). Each core runs an identical program on its slice;
weights are replicated. Host-side prep only reshapes/transposes/packs inputs
into PE-friendly layouts (no FLOPs moved off-device; a constant pre-scale on
weights keeps fp8 encodings out of the subnormal range and is undone
on-device).

Per-core pipeline (L=512, D=256, H=16, QK=V=32, OUT=256):
  1. projections as fp8 DoubleRow matmuls (contract D=256 as 2 planes of
     128): qT/kT = W^T x^T (+ spatial encoding on q), Vaug = [v | pos_CB | 1]
     row-masked by the key mask
  2. per head: S^T = k_h q_h^T in [key, query] layout -> E ~ exp(S^T).
     The 32 [128,1024] exponentials are split across THREE engines: ACT runs
     true exp; DVE/Pool run a Schraudolph fast-exp (int16 bitcast in fp16
     bitspace, ~+-4% ripple; its uniform scale is matched on ACT tiles via an
     ln-scale bias and cancels in the softmax normalization). alpha@Vaug via
     col-tiled matmuls gives feat_node rows, apb rows and the softmax denom.
  3. normalize, spatial features (dist / frame-rotated pts / dir / atan2).
     All ACT work stays on the exp+ln function table (sqrt/rsqrt via
     exp(+-0.5 ln x)): one table load total.
  4. output MLP + residual + layernorm
"""

import os
from contextlib import ExitStack

import numpy as np
import ml_dtypes

import concourse.bass as bass
import concourse.tile as tile
from concourse import bacc
from concourse import mybir
from concourse.bass_utils import run_bass_kernel_spmd
from concourse.masks import make_identity

N, L, D = 8, 512, 256
H, QK, V = 16, 32, 32
OUT = 256
PI = 3.14159265358979323846

f32 = mybir.dt.float32
i16 = mybir.dt.int16
bf16 = mybir.dt.bfloat16
FP8 = mybir.dt.float8e4

AF = mybir.ActivationFunctionType
ALU = mybir.AluOpType
AX = mybir.AxisListType

# fp16 for all non-fp8 matmul operands: 1 cycle/row on the PE, 2x DVE modes
# on 2-byte copies, and enough mantissa for the sign-discontinuous spatial
# features (atan2 / dir).
MMDT = mybir.dt.float16

# fast-exp (Schraudolph in fp16 bitspace): i16(S*EXPA + EXPB) bitcast to fp16
# equals SCL * exp(S) * (1 +- 3.9% ripple). SCL cancels in softmax; ACT-exp
# tiles are biased by ln(SCL) so all tiles share one scale.
EXPA = 1477.3197  # 2^10 / ln 2
EXPB = 17348.8    # 15*2^10 (fp16 exp bias) + 2*2^10 (headroom) - 59.2
LNSC = 1.3854800628848514  # ln(measured SCL) under truncating f32->i16
WS = 16.0  # host weight pre-scale (fp8 subnormal avoidance), undone on-chip
RS = 8.0   # r1 (spatial-encoding hidden) pre-scale


def build_program():
    KGROUPS = int(os.environ.get("KGROUPS", "4"))
    # exp engine schedule: one char per [128,1024] exp tile in emission order
    # (8 early tiles then 24 pipelined), A=ACT true exp, D=DVE, P=Pool
    # PSUM can only be read by ACT ("A") and DVE ("D") -- the BIR verifier
    # rejects GPSIMD-PSUM access -- so every engine string below is A/D only.
    EXPSCHED = os.environ.get("EXPSCHED", "A" * 32)
    QCOPY = os.environ.get("QCOPY", "ADDD")  # qT PSUM->SBUF copy engine by mc
    KCOPY = os.environ.get("KCOPY", "DDDD")
    VENG = os.environ.get("VENG", "DDDD")    # Vaug value-mask engine by lc
    HRELU = os.environ.get("HRELU", "DADA")  # h1 relu engine by mc
    nc = bacc.Bacc()

    def inp(name, shape, dt=f32):
        return nc.declare_dram_parameter(name, list(shape), dt, isOutput=False)

    # packed DRAM inputs: few large DMAs instead of ~48 small ones
    pkQ_d = inp("pkQ", (128, 2048), MMDT)   # xT (2x512) | Wq (2x512)
    pkK_d = inp("pkK", (128, 1024), MMDT)   # Wk
    pkX8_d = inp("pkX8", (128, 1024), FP8)  # xT in fp8 (v DoubleRow lhsT)
    pkV_d = inp("pkV", (128, 1024), FP8)    # WvDR = [Wv0 | Wv1], x WS
    pkW_d = inp("pkW", (128, 3584), MMDT)   # otW1 | otW2
    pkM_d = inp("pkM", (128, 80))           # maskpm | mask16pm | posCB |
                                            # posCA | frame9 | seb2pm |
                                            # seb2x16pm | otb1pm
    pkC_d = inp("pkC", (128, 1792))         # x | otb2B | lngB | lnbB
    pkD_d = inp("pkD", (3, 544), MMDT)      # posCAT | seW1
    seW2_d = inp("seW2", (QK, H * QK), MMDT)
    seb1_d = inp("seb1", (QK, 1))
    out_d = nc.declare_dram_parameter("out", [L, OUT], f32, isOutput=True)

    DR = mybir.MatmulPerfMode.DoubleRow

    with tile.TileContext(nc) as tc, ExitStack() as ctx:
        consts = ctx.enter_context(tc.tile_pool(name="consts", bufs=1))
        wpool = ctx.enter_context(tc.tile_pool(name="weights", bufs=1))
        work = ctx.enter_context(tc.tile_pool(name="work", bufs=1))

        # ---- constants -------------------------------------------------
        ident = consts.tile([128, 128], MMDT)
        make_identity(nc, ident)
        # dummy exp hoists the single ACT table load off the critical path
        dm1 = consts.tile([1, 1], f32)
        nc.vector.memset(dm1, 0.0)
        nc.scalar.activation(out=dm1, in_=dm1, func=AF.Exp)
        e20 = consts.tile([128, 1], f32)
        nc.vector.memset(e20, 1e-20)
        e5 = consts.tile([128, 1], f32)
        nc.vector.memset(e5, 1e-5)
        lnsc = consts.tile([128, 1], f32)
        nc.vector.memset(lnsc, LNSC)

        # ---- input DMAs ------------------------------------------------
        def dma(t, src, eng=None):
            if not isinstance(src, bass.AP):
                src = src[:, :]
            (eng or nc.sync).dma_start(out=t, in_=src)

        pkD = wpool.tile([3, 544], MMDT, name="pkD")
        dma(pkD, pkD_d)
        pkQ = wpool.tile([128, 2048], MMDT, name="pkQ")
        dma(pkQ[:, 0:1024], pkQ_d[:, 0:1024])    # xT (gates everything)
        dma(pkQ[:, 1024:2048], pkQ_d[:, 1024:2048])  # Wq
        pkK = wpool.tile([128, 1024], MMDT, name="pkK")
        dma(pkK, pkK_d)
        seW2s = wpool.tile([32, 512], MMDT)
        dma(seW2s, seW2_d[:, :], eng=nc.gpsimd)
        seb1s = wpool.tile([32, 1], f32)
        dma(seb1s, seb1_d[:, :], eng=nc.gpsimd)
        pkM = wpool.tile([128, 80], f32, name="pkM")
        dma(pkM, pkM_d, eng=nc.gpsimd)
        pkX8 = wpool.tile([128, 1024], FP8, name="pkX8")
        dma(pkX8, pkX8_d)
        pkV = wpool.tile([128, 1024], FP8, name="pkV")
        dma(pkV, pkV_d)
        pkC = wpool.tile([128, 1792], f32, name="pkC")
        dma(pkC, pkC_d)
        pkW = wpool.tile([128, 3584], MMDT, name="pkW")
        dma(pkW, pkW_d)  # otW1 | otW2 (needed last)

        xTs = [pkQ[:, 512 * i : 512 * (i + 1)] for i in range(2)]
        Wqs = [pkQ[:, 1024 + 512 * i : 1024 + 512 * (i + 1)] for i in range(2)]
        Wks = [pkK[:, 512 * i : 512 * (i + 1)] for i in range(2)]
        x8dr = pkX8[:, :].rearrange("p (two n) -> p two n", two=2)
        wvdr = pkV[:, :].rearrange("p (two n) -> p two n", two=2)
        otW1s = [pkW[:, 512 * i : 512 * (i + 1)] for i in range(5)]
        otW2s = [pkW[:, 2560 + 256 * i : 2560 + 256 * (i + 1)] for i in range(4)]
        posCATs = pkD[:, 0:512]
        seW1s = pkD[:, 512:544]
        xn = [pkC[:, 256 * c : 256 * (c + 1)] for c in range(4)]
        otb2B = pkC[:, 1024:1280]
        lngB = pkC[:, 1280:1536]
        lnbB = pkC[:, 1536:1792]
        maskpm = pkM[:, 0:4]
        mask16pm = pkM[:, 4:8]
        posCBn = [pkM[:, 8 + 3 * c : 8 + 3 * (c + 1)] for c in range(4)]
        frame_sec = 32  # frame9 cols at 32 + 9c .. 68
        seb2s = pkM[:, 68:72]
        seb2x16 = pkM[:, 72:76]
        otb1s = pkM[:, 76:80]
        maskr = wpool.tile([128, 4], MMDT)
        nc.vector.tensor_copy(out=maskr, in_=maskpm)
        # frame columns broadcast over heads, hoisted off the spatial-phase
        # critical chain (Pool is idle during projections)
        fbA = wpool.tile([128, 576], f32, name="fbA")
        fbv = fbA.rearrange("p (k c h) -> p k c h", c=4, h=16)
        for k in range(9):
            nc.gpsimd.tensor_copy(
                out=fbv[:, k],
                in_=pkM[:, frame_sec + k : frame_sec + 36 : 9].unsqueeze(2)
                .broadcast_to([128, 4, 16]),
            )

        def _veng(c):
            assert c == "D", f"PSUM reader must be A or D, got {c}"
            return nc.vector

        exp_idx = [0]

        def make_E(ps_S, pool, name, tag=None, width=512):
            c = EXPSCHED[exp_idx[0] % len(EXPSCHED)]
            exp_idx[0] += 1
            E = pool.tile([128, width], MMDT, tag=tag, name=name)
            if c == "A":
                nc.scalar.activation(out=E, in_=ps_S, func=AF.Exp,
                                     bias=lnsc, scale=1.0)
            else:
                _veng(c).tensor_scalar(
                    out=E.bitcast(i16), in0=ps_S, scalar1=EXPA, scalar2=EXPB,
                    op0=ALU.mult, op1=ALU.add,
                )
            return E

        # ---- projections ----------------------------------------------
        qT, kT, Vaug = [], [], []
        with tc.tile_pool(name="psproj", bufs=1, space="PSUM") as psproj:
            # spatial encoding: r1 = relu(se_W1^T @ posCA^T + b1)
            ps_r1 = psproj.tile([128, 512], f32, tag="ps_v0", name="ps_r1")
            nc.tensor.matmul(out=ps_r1[0:32, :], lhsT=seW1s, rhs=posCATs,
                             start=True, stop=True)
            r1 = work.tile([32, 512], MMDT, name="r1")
            nc.scalar.activation(out=r1, in_=ps_r1[0:32, :], func=AF.Relu,
                                 bias=seb1s, scale=1.0)

            def emit_qk(mc):
                # k first: it does not wait on the r1/spat chain, so kT[mc]
                # lands early and the first S matmuls are q-gated only
                ps_k = psproj.tile([128, 512], f32, tag="ps_k")
                for kc in range(2):
                    nc.tensor.matmul(
                        out=ps_k,
                        lhsT=Wks[kc][:, 128 * mc : 128 * (mc + 1)],
                        rhs=xTs[kc], start=(kc == 0), stop=(kc == 1),
                    )
                t = work.tile([128, 512], MMDT, name=f"kT{mc}")
                c = KCOPY[mc]
                if c == "A":
                    nc.scalar.activation(out=t, in_=ps_k, func=AF.Copy)
                else:
                    nc.vector.tensor_copy(out=t, in_=ps_k)
                kT.append(t)

                ps_q = psproj.tile([128, 512], f32, tag="ps_q")
                for kc in range(2):
                    nc.tensor.matmul(
                        out=ps_q,
                        lhsT=Wqs[kc][:, 128 * mc : 128 * (mc + 1)],
                        rhs=xTs[kc], start=(kc == 0), stop=False,
                    )
                nc.tensor.matmul(
                    out=ps_q, lhsT=seW2s[:, 128 * mc : 128 * (mc + 1)],
                    rhs=r1, start=False, stop=True,
                )
                t = work.tile([128, 512], MMDT, name=f"qT{mc}")
                c = QCOPY[mc]
                if c == "A":
                    nc.scalar.activation(
                        out=t, in_=ps_q, func=AF.Identity,
                        bias=seb2s[:, mc : mc + 1], scale=1.0,
                    )
                else:
                    _veng(c).tensor_scalar(
                        out=t, in0=ps_q, scalar1=seb2s[:, mc : mc + 1],
                        scalar2=None, op0=ALU.add,
                    )
                qT.append(t)

            emit_qk(0)
            # heads 0-3 S+exp emitted mid-projections (they only need the
            # mc=0 chunk of qT/kT): the exp streams start ~6us earlier
            earlyEs = {}

            def emit_early(eh):
                ch, r = eh // 4, (eh % 4) * 32
                Es = []
                for half in range(2):
                    ps_S = psproj.tile(
                        [128, 1024], f32, tag=f"ps_S{half}",
                        name=f"psSe{eh}_{half}",
                    )
                    for kcl in range(2):
                        kc = 2 * half + kcl
                        nc.tensor.matmul(
                            out=ps_S[:, 512 * kcl : 512 * (kcl + 1)],
                            lhsT=kT[ch][r : r + 32,
                                        128 * kc : 128 * (kc + 1)],
                            rhs=qT[ch][r : r + 32, :],
                            start=True,
                            stop=True,
                            tile_position=(r, 0),
                        )
                    Es.append(make_E(ps_S, work, f"Ee{eh}_{half}", width=1024))
                earlyEs[eh] = Es

            emit_early(0)
            emit_qk(1)
            emit_early(1)

            for lc in range(4):
                ps_v = psproj.tile([128, 512], f32, tag=f"ps_v{lc % 2}")
                nc.tensor.matmul(
                    out=ps_v, lhsT=x8dr[:, :, 128 * lc : 128 * (lc + 1)],
                    rhs=wvdr, start=True, stop=True, perf_mode=DR,
                )
                va = work.tile([128, H * 36], MMDT, name=f"Vaug{lc}")
                vav = va.rearrange("p (h j) -> p h j", j=36)
                mcol = maskpm[:, lc : lc + 1]
                m16col = mask16pm[:, lc : lc + 1]
                c = VENG[lc]
                # value columns: (WS*v) * mask/WS  (kills the -INF logit
                # bias: masked keys contribute 0 to numerator and denom)
                if c == "A":
                    nc.scalar.activation(
                        out=vav[:, :, 0:32],
                        in_=ps_v.rearrange("p (h j) -> p h j", j=32),
                        func=AF.Copy, scale=m16col,
                    )
                else:
                    _veng(c).tensor_scalar(
                        out=vav[:, :, 0:32],
                        in0=ps_v.rearrange("p (h j) -> p h j", j=32),
                        scalar1=m16col, scalar2=None, op0=ALU.mult,
                    )
                # pos_CB columns (masked), replicated across heads
                pcbm = work.tile([128, 3], MMDT, tag="pcbm")
                nc.gpsimd.tensor_scalar(
                    out=pcbm, in0=posCBn[lc], scalar1=mcol, scalar2=None,
                    op0=ALU.mult,
                )
                nc.gpsimd.tensor_copy(
                    out=vav[:, :, 32:35],
                    in_=pcbm.unsqueeze(1).broadcast_to([128, H, 3]),
                )
                # ones column -> softmax denominator (masked)
                nc.gpsimd.tensor_copy(
                    out=vav[:, :, 35:36],
                    in_=maskr[:, lc : lc + 1].unsqueeze(1)
                    .broadcast_to([128, H, 1]),
                )
                Vaug.append(va)
                if lc == 0:
                    emit_early(2)
                elif lc == 1:
                    emit_early(3)
            emit_qk(2)
            emit_qk(3)

        # ---- attention -------------------------------------------------
        # Per head h: S^T = k_h q_h^T in [key, query] chunks, E ~ exp(S^T)
        # in fp16, then FLIPPED value matmuls: E chunks stationary, Vaug36
        # columns streaming so feat_node, apb and the softmax denominator
        # land in [query, feat] layout; normalization is per-partition.
        featT = [work.tile([128, 512], MMDT, name=f"featT{g}") for g in range(5)]
        featA = work.tile([128, 2048], MMDT, name="featA")
        apbA = work.tile([128, 256], f32, name="apbA")
        recALL = work.tile([128, 64], f32, name="recALL")  # [l, 16c+4g+s]

        with (
            tc.tile_pool(name="psS", bufs=int(os.environ.get("PSSBUFS", "3")), space="PSUM") as psS_pool,
            tc.tile_pool(name="psFU", bufs=1, space="PSUM") as psFU_pool,
            tc.tile_pool(name="Epool", bufs=6) as E_pool,
            tc.tile_pool(name="gwork", bufs=4) as gwork,
        ):
            psFUgs = {}
            psFUtiles = {}

            def emit_S(h):
                ch, r = h // 4, (h % 4) * 32
                Es = []
                for half in range(2):
                    ps_S = psS_pool.tile([128, 1024], f32, tag="ps_S")
                    for kcl in range(2):
                        kc = 2 * half + kcl
                        nc.tensor.matmul(
                            out=ps_S[:, 512 * kcl : 512 * (kcl + 1)],
                            lhsT=kT[ch][r : r + 32, 128 * kc : 128 * (kc + 1)],
                            rhs=qT[ch][r : r + 32, :],
                            start=True,
                            stop=True,
                            tile_position=(r, 0),
                        )
                    Es.append(make_E(ps_S, E_pool, f"E{h}_{half}", tag="E",
                                     width=1024))
                return Es

            def emit_AV(h, Es):
                g, s = h // 4, h % 4
                if s == 0:
                    psFUa = psFU_pool.tile([128, 288], f32, tag="psFUa",
                                           name=f"psFUa_{g}")
                    psFUb = psFU_pool.tile([128, 288], f32, tag="psFUb",
                                           name=f"psFUb_{g}")
                    psFUtiles[g] = (psFUa, psFUb)
                    psFUgs[g] = [
                        (psFUa, psFUb)[c // 2][
                            :, 144 * (c % 2) : 144 * (c % 2 + 1)]
                        for c in range(4)
                    ]
                psFUg = psFUgs[g]
                for c in range(4):
                    for kc in range(4):
                        nc.tensor.matmul(
                            out=psFUg[c][:, 36 * s : 36 * (s + 1)],
                            lhsT=Es[kc // 2][
                                :,
                                512 * (kc % 2) + 128 * c :
                                512 * (kc % 2) + 128 * (c + 1),
                            ],
                            rhs=Vaug[kc][:, 36 * h : 36 * (h + 1)],
                            start=(kc == 0),
                            stop=(kc == 3),
                        )

            A4 = apbA.rearrange("p (c h j) -> p c h j", h=16, j=4)
            RC = recALL.rearrange("p (c h) -> p c h", h=16)
            FA = featA.rearrange("p (c g2 s j) -> p c g2 s j", g2=4, s=4, j=32)

            # spatial stage-1 (no ACT): apb normalize + frame-rotated pts +
            # square-sums, emitted per head-half as soon as that half's
            # groups finish -- DVE/Pool are idle during the ACT exp stream
            recB4 = work.tile([128, 256], f32, name="recB4")
            rb4 = recB4.rearrange("p (c h j) -> p c h j", h=16, j=4)
            spatA = work.tile([128, 512], MMDT, name="spatA")
            spatc = spatA.rearrange("p (c f) -> p c f", f=128)
            ptsF = work.tile([128, 192], f32, name="ptsF")
            ptsv = ptsF.rearrange("p (c h i) -> p c h i", h=16, i=3)
            sqA = work.tile([128, 192], f32, name="sqA")
            sqav = sqA.rearrange("p (c h i) -> p c h i", h=16, i=3)
            sqB = work.tile([128, 192], f32, name="sqB")
            sqbv = sqB.rearrange("p (c h i) -> p c h i", h=16, i=3)
            tDistA = work.tile([128, 64], f32, name="tDistA")
            tDistv = tDistA.rearrange("p (c h) -> p c h", h=16)
            tDirA = work.tile([128, 64], f32, name="tDirA")
            tDirv = tDirA.rearrange("p (c h) -> p c h", h=16)
            spT1 = work.tile([128, 64], f32, name="spT1")
            spT1v = spT1.rearrange("p (c h) -> p c h", h=16)
            spT2 = work.tile([128, 64], f32, name="spT2")
            spT2v = spT2.rearrange("p (c h) -> p c h", h=16)
            spT3 = work.tile([128, 64], f32, name="spT3")
            spT3v = spT3.rearrange("p (c h) -> p c h", h=16)
            spT4 = work.tile([128, 64], f32, name="spT4")
            spT4v = spT4.rearrange("p (c h) -> p c h", h=16)

            def spatial_stage1(h0, h1):
                n = h1 - h0
                hs = slice(h0, h1)
                A4h = A4[:, :, hs, :]
                rbh = rb4[:, :, hs, :]
                nc.vector.tensor_copy(
                    out=rbh,
                    in_=RC[:, :, hs].unsqueeze(3).broadcast_to([128, 4, n, 4]),
                )
                nc.vector.tensor_tensor(out=A4h, in0=A4h, in1=rbh, op=ALU.mult)
                nc.vector.tensor_tensor(
                    out=A4h[:, :, :, 0:3], in0=A4h[:, :, :, 0:3],
                    in1=pkM[:, 20:32].rearrange("p (c j) -> p c j", j=3)
                    .unsqueeze(2).broadcast_to([128, 4, n, 3]),
                    op=ALU.subtract,
                )
                for i in range(3):
                    eng = nc.vector if i == 1 else nc.gpsimd
                    ta, tb = (spT1v, spT2v) if i == 1 else (spT3v, spT4v)
                    ta, tb = ta[:, :, hs], tb[:, :, hs]
                    eng.tensor_tensor(
                        out=ta, in0=A4h[:, :, :, 0],
                        in1=fbv[:, 3 * i + 0][:, :, hs], op=ALU.mult,
                    )
                    eng.tensor_tensor(
                        out=tb, in0=A4h[:, :, :, 1],
                        in1=fbv[:, 3 * i + 1][:, :, hs], op=ALU.mult,
                    )
                    eng.tensor_tensor(out=ta, in0=ta, in1=tb, op=ALU.add)
                    eng.tensor_tensor(
                        out=tb, in0=A4h[:, :, :, 2],
                        in1=fbv[:, 3 * i + 2][:, :, hs], op=ALU.mult,
                    )
                    eng.tensor_tensor(
                        out=ptsv[:, :, hs, i], in0=ta, in1=tb, op=ALU.add
                    )
                nc.vector.tensor_copy(
                    out=spatc[:, :, 3 * h0 : 3 * h1],
                    in_=ptsF.rearrange("p (c f) -> p c f", f=48)
                    [:, :, 3 * h0 : 3 * h1],
                )
                nc.gpsimd.tensor_tensor(
                    out=sqav[:, :, hs, :], in0=A4h[:, :, :, 0:3],
                    in1=A4h[:, :, :, 0:3], op=ALU.mult,
                )
                nc.vector.tensor_reduce(
                    out=tDistv[:, :, hs], in_=sqav[:, :, hs, :],
                    axis=AX.X, op=ALU.add,
                )
                nc.gpsimd.tensor_tensor(
                    out=sqbv[:, :, hs, :], in0=ptsv[:, :, hs, :],
                    in1=ptsv[:, :, hs, :], op=ALU.mult,
                )
                nc.vector.tensor_reduce(
                    out=tDirv[:, :, hs], in_=sqbv[:, :, hs, :],
                    axis=AX.X, op=ALU.add,
                )

            def emit_post(g):
                # apb + denom rows out of PSUM (2 merged copies), merged
                # per-query reciprocal, one Pool broadcast, then the
                # normalization fused into the PSUM->SBUF feat copy
                psFUa, psFUb = psFUtiles[g]
                for ca, pst in enumerate((psFUa, psFUb)):
                    nc.vector.tensor_copy(
                        out=A4[:, 2 * ca : 2 * ca + 2, 4 * g : 4 * g + 4, :],
                        in_=pst.rearrange("p (c s j) -> p c s j", s=4, j=36)
                        [:, :, :, 32:36],
                    )
                nc.vector.reciprocal(
                    out=RC[:, :, 4 * g : 4 * g + 4],
                    in_=A4[:, :, 4 * g : 4 * g + 4, 3],
                )
                bcg = gwork.tile([128, 512], f32, tag="bcg")
                bcv = bcg.rearrange("p (c s j) -> p c s j", s=4, j=32)
                nc.vector.tensor_copy(
                    out=bcv,
                    in_=RC[:, :, 4 * g : 4 * g + 4].unsqueeze(3)
                    .broadcast_to([128, 4, 4, 32]),
                )
                for ca, pst in enumerate((psFUa, psFUb)):
                    nc.vector.tensor_tensor(
                        out=FA[:, 2 * ca : 2 * ca + 2, g, :, :],
                        in0=pst.rearrange("p (c s j) -> p c s j", s=4, j=36)
                        [:, :, :, 0:32],
                        in1=bcv[:, 2 * ca : 2 * ca + 2, :, :],
                        op=ALU.mult,
                    )

            def emit_post_tr(g):
                # feat_node transpose for group g: [q, feat] -> [feat, q];
                # output borrows a rotating ps_S slot (bitcast to fp16) so no
                # dedicated PSUM bank is needed
                psFT_raw = psS_pool.tile([128, 1024], f32, tag="ps_S",
                                         name=f"psFT{g}")
                psFT = psFT_raw.bitcast(MMDT)[:, 0:512]
                for c in range(4):
                    nc.tensor.transpose(
                        out=psFT[:, 128 * c : 128 * (c + 1)],
                        in_=featA[:, 512 * c + 128 * g : 512 * c + 128 * g + 128],
                        identity=ident,
                    )
                nc.vector.tensor_copy(out=featT[g], in_=psFT)

            # software-pipelined: S(h+1) is emitted BEFORE AV(h) so the
            # in-order PE stream computes the next head's logits during
            # exp(h) instead of stalling behind AV matmuls waiting on E(h)
            KLAG = int(os.environ.get("KLAG", "4"))
            pend = []

            def drain_one():
                hp, Esp = pend.pop(0)
                emit_AV(hp, Esp)
                if hp % 4 == 3:
                    emit_post(hp // 4)
                    if hp // 4 == 1:
                        spatial_stage1(0, 8)
                if hp % 4 == 2 and hp // 4 > 0:
                    emit_post_tr(hp // 4 - 1)

            for h in range(4 * KGROUPS):
                Es = earlyEs.pop(h) if h in earlyEs else emit_S(h)
                pend.append((h, Es))
                if len(pend) > KLAG:
                    drain_one()
            while pend:
                drain_one()
            emit_post_tr(KGROUPS - 2)
            emit_post_tr(KGROUPS - 1)
            spatial_stage1(8, 16)

        with tc.tile_pool(name="spwork", bufs=1) as spw:
            tA = spw.tile([128, 64], f32, name="tA")
            tB = spw.tile([128, 64], f32, name="tB")
            tAv = tA.rearrange("p (c h) -> p c h", h=16)
            tBv = tB.rearrange("p (c h) -> p c h", h=16)
            axM = spw.tile([128, 64], f32, name="axM")
            ayM = spw.tile([128, 64], f32, name="ayM")
            qsM = spw.tile([128, 64], f32, name="qsM")
            axv = axM.rearrange("p (c h) -> p c h", h=16)
            ayv = ayM.rearrange("p (c h) -> p c h", h=16)
            qsv = qsM.rearrange("p (c h) -> p c h", h=16)
            a2 = spw.tile([128, 64], f32, name="a2")
            a2v = a2.rearrange("p (c h) -> p c h", h=16)
            a4 = spw.tile([128, 64], f32, name="a4")
            a4v = a4.rearrange("p (c h) -> p c h", h=16)
            pA = spw.tile([128, 64], f32, name="pA")
            pAv = pA.rearrange("p (c h) -> p c h", h=16)
            pB = spw.tile([128, 64], f32, name="pB")
            pBv = pB.rearrange("p (c h) -> p c h", h=16)
            usM = spw.tile([128, 64], f32, name="usM")
            usv = usM.rearrange("p (c h) -> p c h", h=16)
            wsM = spw.tile([128, 64], f32, name="wsM")
            wsv = wsM.rearrange("p (c h) -> p c h", h=16)
            sM = spw.tile([128, 64], f32, name="sM")
            sv = sM.rearrange("p (c h) -> p c h", h=16)
            u2M = spw.tile([128, 64], f32, name="u2M")
            u2v = u2M.rearrange("p (c h) -> p c h", h=16)

            def spatial_stage2(h0, h1):
                # sqrt/dir/atan for one head-half; half 0-7 runs while half
                # 8-15's stage-1 is still on DVE/Pool
                n = h1 - h0
                hs = slice(h0, h1)
                # dist (first Sqrt after the last exp -> one table switch)
                nc.scalar.activation(
                    out=spatc[:, :, 48 + h0 : 48 + h1], in_=tDistv[:, :, hs],
                    func=AF.Sqrt, bias=e20,
                )
                # dir = pts / sqrt(p2 + tiny)
                nc.scalar.activation(out=tBv[:, :, hs], in_=tDirv[:, :, hs],
                                     func=AF.Sqrt, bias=e20)
                nc.vector.reciprocal(out=tBv[:, :, hs], in_=tBv[:, :, hs])
                nc.gpsimd.tensor_tensor(
                    out=spatc[:, :, 64 + 3 * h0 : 64 + 3 * h1]
                    .rearrange("p c (h i) -> p c h i", i=3),
                    in0=ptsv[:, :, hs, :],
                    in1=tBv[:, :, hs].unsqueeze(3)
                    .broadcast_to([128, 4, n, 3]),
                    op=ALU.mult,
                )
                # atan2 range reduction: a = min(|x|,|y|) / max(|x|,|y|)
                pxh, pyh = ptsv[:, :, hs, 0], ptsv[:, :, hs, 1]
                axh, ayh = axv[:, :, hs], ayv[:, :, hs]
                qsh = qsv[:, :, hs]
                tAh, tBh = tAv[:, :, hs], tBv[:, :, hs]
                nc.scalar.activation(out=axh, in_=pxh, func=AF.Abs)
                nc.scalar.activation(out=ayh, in_=pyh, func=AF.Abs)
                nc.vector.tensor_tensor(out=tAh, in0=axh, in1=ayh, op=ALU.min)
                nc.vector.scalar_tensor_tensor(
                    out=tBh, in0=axh, scalar=1e-38, in1=ayh,
                    op0=ALU.add, op1=ALU.max,
                )
                nc.vector.reciprocal(out=tBh, in_=tBh)
                nc.gpsimd.tensor_tensor(out=qsh, in0=tAh, in1=tBh, op=ALU.mult)
                ang = spatc[:, :, 112 + h0 : 112 + h1]
                # masks / sign on DVE (parallel with the Pool poly chain):
                #   atan2 = P(q)*q*us + ws,  us = (1-2m1)(1-2m2)*s,
                #   ws = (m1*(pi/2)*(1-2m2) + m2*pi)*s
                svh = sv[:, :, hs]
                u2h = u2v[:, :, hs]
                ush = usv[:, :, hs]
                wsh = wsv[:, :, hs]
                nc.vector.tensor_tensor(out=tAh, in0=ayh, in1=axh, op=ALU.is_gt)
                nc.vector.tensor_scalar(
                    out=tBh, in0=pxh, scalar1=0.0, scalar2=None, op0=ALU.is_lt
                )
                nc.vector.tensor_scalar(
                    out=svh, in0=pyh, scalar1=0.0, scalar2=2.0,
                    op0=ALU.is_ge, op1=ALU.mult,
                )
                nc.vector.tensor_scalar(
                    out=svh, in0=svh, scalar1=-1.0, scalar2=None, op0=ALU.add
                )
                nc.vector.tensor_scalar(
                    out=u2h, in0=tBh, scalar1=-2.0, scalar2=1.0,
                    op0=ALU.mult, op1=ALU.add,
                )
                nc.vector.tensor_scalar(
                    out=ush, in0=tAh, scalar1=-2.0, scalar2=1.0,
                    op0=ALU.mult, op1=ALU.add,
                )
                nc.vector.tensor_tensor(out=ush, in0=ush, in1=u2h, op=ALU.mult)
                nc.vector.scalar_tensor_tensor(
                    out=wsh, in0=tAh, scalar=PI / 2, in1=u2h,
                    op0=ALU.mult, op1=ALU.mult,
                )
                nc.vector.scalar_tensor_tensor(
                    out=wsh, in0=tBh, scalar=PI, in1=wsh,
                    op0=ALU.mult, op1=ALU.add,
                )
                nc.vector.tensor_tensor(out=ush, in0=ush, in1=svh, op=ALU.mult)
                nc.vector.tensor_tensor(out=wsh, in0=wsh, in1=svh, op=ALU.mult)
                # Estrin poly on Pool
                a2h, a4h = a2v[:, :, hs], a4v[:, :, hs]
                pAh, pBh = pAv[:, :, hs], pBv[:, :, hs]
                nc.gpsimd.tensor_tensor(out=a2h, in0=qsh, in1=qsh, op=ALU.mult)
                nc.gpsimd.tensor_scalar(
                    out=pAh, in0=a2h, scalar1=-0.0851330, scalar2=0.1801410,
                    op0=ALU.mult, op1=ALU.add,
                )
                nc.gpsimd.tensor_scalar(
                    out=pBh, in0=a2h, scalar1=-0.3302995, scalar2=0.9998660,
                    op0=ALU.mult, op1=ALU.add,
                )
                nc.gpsimd.tensor_tensor(out=a4h, in0=a2h, in1=a2h, op=ALU.mult)
                nc.gpsimd.tensor_tensor(out=pAh, in0=pAh, in1=a4h, op=ALU.mult)
                nc.gpsimd.tensor_tensor(out=pAh, in0=pAh, in1=pBh, op=ALU.add)
                nc.gpsimd.tensor_tensor(out=pAh, in0=pAh, in1=qsh, op=ALU.mult)
                nc.vector.tensor_tensor(out=pAh, in0=pAh, in1=ush, op=ALU.mult)
                nc.vector.tensor_tensor(out=ang, in0=pAh, in1=wsh, op=ALU.add)

            spatial_stage2(0, 8)
            spatial_stage2(8, 16)

        # ---- MLP part 1: h1 partial accumulation over feat_node ---------
        xb = []
        for c in range(4):
            t = work.tile([128, 256], f32, name=f"xb{c}")
            nc.gpsimd.tensor_scalar(
                out=t, in0=otb2B, scalar1=maskpm[:, c : c + 1], scalar2=None,
                op0=ALU.mult,
            )
            nc.gpsimd.tensor_tensor(out=t, in0=t, in1=xn[c], op=ALU.add)
            xb.append(t)
        h1T = [work.tile([128, 512], MMDT, name=f"h1T{mc}") for mc in range(4)]
        psM_pool = ctx.enter_context(tc.tile_pool(name="psM", bufs=4, space="PSUM"))
        ps_hs = []
        for mc in range(4):
            ps_h = psM_pool.tile([128, 512], f32, tag="ps_h", name=f"ps_h{mc}")
            ps_hs.append(ps_h)
            for kc in range(4):
                nc.tensor.matmul(
                    out=ps_h,
                    lhsT=otW1s[kc][:, 128 * mc : 128 * (mc + 1)],
                    rhs=featT[kc],
                    start=(kc == 0),
                    stop=False,
                )
        # ---- spatial features, post-attention remainder (ACT sqrt parts
        # and atan2; normalize/pts/square-sums ran in spatial_stage1) ------

        # transpose spatial features into featT[4]
        with tc.tile_pool(name="psSp", bufs=1, space="PSUM") as psSp_pool:
            ps_sp = psSp_pool.tile([128, 512], MMDT)
            for c in range(4):
                nc.tensor.transpose(
                    out=ps_sp[:, 128 * c : 128 * (c + 1)],
                    in_=spatA[:, 128 * c : 128 * (c + 1)],
                    identity=ident,
                )
            nc.vector.tensor_copy(out=featT[4], in_=ps_sp)

        # ---- MLP part 2: kc=4 finals, relu, out-stage, layernorm --------
        with tc.tile_pool(name="psO", bufs=1, space="PSUM") as psO_pool:
            for mc in range(4):
                nc.tensor.matmul(
                    out=ps_hs[mc],
                    lhsT=otW1s[4][:, 128 * mc : 128 * (mc + 1)],
                    rhs=featT[4],
                    start=False,
                    stop=True,
                )
                if HRELU[mc] == "A":
                    nc.scalar.activation(
                        out=h1T[mc], in_=ps_hs[mc], func=AF.Relu,
                        bias=otb1s[:, mc : mc + 1], scale=1.0,
                    )
                else:
                    nc.vector.tensor_scalar(
                        out=h1T[mc], in0=ps_hs[mc],
                        scalar1=otb1s[:, mc : mc + 1], scalar2=0.0,
                        op0=ALU.add, op1=ALU.max,
                    )
            ps_os = [
                psO_pool.tile([128, 256], f32, tag=f"ps_o{lc}", name=f"ps_o{lc}")
                for lc in range(4)
            ]
            for lc in range(4):
                for kc in range(4):
                    nc.tensor.matmul(
                        out=ps_os[lc],
                        lhsT=h1T[kc][:, 128 * lc : 128 * (lc + 1)],
                        rhs=otW2s[kc],
                        start=(kc == 0),
                        stop=(kc == 3),
                    )
            for lc in range(4):
                y = work.tile([128, 256], f32, name=f"y_ln{lc}")
                if lc % 2 == 0:
                    nc.vector.scalar_tensor_tensor(
                        out=y, in0=ps_os[lc], scalar=maskpm[:, lc : lc + 1],
                        in1=xb[lc], op0=ALU.mult, op1=ALU.add,
                    )
                else:
                    t = work.tile([128, 256], f32, name=f"yt{lc}")
                    nc.scalar.activation(
                        out=t, in_=ps_os[lc], func=AF.Copy,
                        scale=maskpm[:, lc : lc + 1],
                    )
                    nc.gpsimd.tensor_tensor(out=y, in0=t, in1=xb[lc],
                                            op=ALU.add)
                stats = work.tile([128, 6], f32, name=f"ln_stats{lc}")
                mv = work.tile([128, 2], f32, name=f"ln_mv{lc}")
                nc.vector.bn_stats(out=stats, in_=y)
                nc.vector.bn_aggr(out=mv, in_=stats)
                mu = mv[:, 0:1]
                rstd = work.tile([128, 1], f32, name=f"ln_rstd{lc}")
                nc.scalar.activation(out=rstd, in_=mv[:, 1:2], func=AF.Sqrt,
                                     bias=e5)
                nc.vector.reciprocal(out=rstd, in_=rstd)
                z = work.tile([128, 256], f32, name=f"z_out{lc}")
                if lc % 2 == 0:
                    nc.vector.scalar_tensor_tensor(
                        out=z, in0=y, scalar=mu, in1=lngB,
                        op0=ALU.subtract, op1=ALU.mult,
                    )
                    nc.vector.scalar_tensor_tensor(
                        out=z, in0=z, scalar=rstd, in1=lnbB,
                        op0=ALU.mult, op1=ALU.add,
                    )
                else:
                    nc.gpsimd.tensor_scalar(
                        out=z, in0=y, scalar1=mu, scalar2=rstd,
                        op0=ALU.subtract, op1=ALU.mult,
                    )
                    nc.gpsimd.tensor_tensor(out=z, in0=z, in1=lngB, op=ALU.mult)
                    nc.gpsimd.tensor_tensor(out=z, in0=z, in1=lnbB, op=ALU.add)
                nc.sync.dma_start(out=out_d[128 * lc : 128 * (lc + 1), :], in_=z)

    nc.finalize()
    return nc


_cached = {}


def _get_program():
    if "nc" not in _cached:
        _cached["nc"] = build_program()
    return _cached["nc"]


def _prep_core_inputs(x, pos_CA, pos_CB, frame, mask, Wq, Wk, Wv, seW1, seb1,
                      seW2, seb2, otW1, otb1, otW2, otb2, lng, lnb):
    f = np.float32
    h16 = np.float16
    f8 = ml_dtypes.float8_e4m3
    xT = x.T.astype(f)                                   # (256, 512)

    Wv16 = (Wv * WS).astype(f)
    pkQ = np.concatenate(
        [xT[0:128], xT[128:256], Wq[0:128], Wq[128:256]], axis=1).astype(f)
    pkK = np.concatenate([Wk[0:128], Wk[128:256]], axis=1).astype(f)
    pkX8 = np.concatenate([xT[0:128], xT[128:256]], axis=1)
    pkV = np.concatenate([Wv16[0:128], Wv16[128:256]], axis=1)
    pkW = np.concatenate(
        [otW1[128 * i : 128 * (i + 1)] for i in range(5)]
        + [otW2[128 * i : 128 * (i + 1)] for i in range(4)], axis=1).astype(f)
    maskf = mask.astype(f).reshape(4, 128).T
    pkM = np.concatenate(
        [maskf, maskf / WS]
        + [pos_CB.astype(f).reshape(4, 128, 3).transpose(1, 0, 2).reshape(128, 12)]
        + [pos_CA.astype(f).reshape(4, 128, 3).transpose(1, 0, 2).reshape(128, 12)]
        + [frame.astype(f).reshape(4, 128, 9).transpose(1, 0, 2).reshape(128, 36)]
        + [seb2.astype(f).reshape(4, 128).T,
           (seb2 * WS).astype(f).reshape(4, 128).T,
           otb1.astype(f).reshape(4, 128).T], axis=1).astype(f)
    xr = x.astype(f).reshape(4, 128, D)
    pkC = np.concatenate(
        [xr[c] for c in range(4)]
        + [np.tile(otb2.astype(f), (128, 1)),
           np.tile(lng.astype(f), (128, 1)),
           np.tile(lnb.astype(f), (128, 1))], axis=1).astype(f)
    pkD = np.concatenate([pos_CA.T.astype(f), seW1.astype(f)], axis=1)
    return {
        "pkQ": np.ascontiguousarray(pkQ.astype(h16)),
        "pkK": np.ascontiguousarray(pkK.astype(h16)),
        "pkX8": np.ascontiguousarray(pkX8.astype(f8)),
        "pkV": np.ascontiguousarray(pkV.astype(f8)),
        "pkW": np.ascontiguousarray(pkW.astype(h16)),
        "pkM": np.ascontiguousarray(pkM),
        "pkC": np.ascontiguousarray(pkC),
        "pkD": np.ascontiguousarray(pkD.astype(h16)),
        "seW2": np.ascontiguousarray(seW2.astype(h16)),
        "seb1": np.ascontiguousarray(seb1.reshape(QK, 1), dtype=f),
    }


def kernel(x, pos_CA, pos_CB, frame, mask,
           Wq, Wk, Wv, se_W1, se_b1, se_W2, se_b2,
           ot_W1, ot_b1, ot_W2, ot_b2, ln_g, ln_b):
    x = np.asarray(x, dtype=np.float32)
    pos_CA = np.asarray(pos_CA, dtype=np.float32)
    pos_CB = np.asarray(pos_CB, dtype=np.float32)
    frame = np.asarray(frame, dtype=np.float32)
    mask = np.asarray(mask)
    weights = [np.asarray(w, dtype=np.float32) for w in
               (Wq, Wk, Wv, se_W1, se_b1, se_W2, se_b2,
                ot_W1, ot_b1, ot_W2, ot_b2, ln_g, ln_b)]

    nc = _get_program()
    in_maps = [
        _prep_core_inputs(x[i], pos_CA[i], pos_CB[i], frame[i], mask[i], *weights)
        for i in range(N)
    ]
    res = run_bass_kernel_spmd(nc, in_maps, list(range(N)))
    _cached["last_results"] = res
    out = np.stack([res.results[i]["out"] for i in range(N)], axis=0)
    return out.astype(np.float32)


# revision 40
# speedup vs baseline: 1.0082x; 1.0082x over previous
"""Trainium2 Bass kernel for nn_EnhancedDDGAttention.

Sharding: data-parallel over the batch axis N=8 -> one batch element per
NeuronCore (8 cores). Each core runs an identical program on its slice;
weights are replicated. Host-side prep only reshapes/transposes/packs inputs
into PE-friendly layouts (no FLOPs moved off-device; a constant pre-scale on
weights keeps fp8 encodings out of the subnormal range and is undone
on-device).

Per-core pipeline (L=512, D=256, H=16, QK=V=32, OUT=256):
  1. projections as fp8 DoubleRow matmuls (contract D=256 as 2 planes of
     128): qT/kT = W^T x^T (+ spatial encoding on q), Vaug = [v | pos_CB | 1]
     row-masked by the key mask
  2. per head: S^T = k_h q_h^T in [key, query] layout -> E ~ exp(S^T).
     The 32 [128,1024] exponentials are split across THREE engines: ACT runs
     true exp; DVE/Pool run a Schraudolph fast-exp (int16 bitcast in fp16
     bitspace, ~+-4% ripple; its uniform scale is matched on ACT tiles via an
     ln-scale bias and cancels in the softmax normalization). alpha@Vaug via
     col-tiled matmuls gives feat_node rows, apb rows and the softmax denom.
  3. normalize, spatial features (dist / frame-rotated pts / dir / atan2).
     All ACT work stays on the exp+ln function table (sqrt/rsqrt via
     exp(+-0.5 ln x)): one table load total.
  4. output MLP + residual + layernorm
"""

import os
from contextlib import ExitStack

import numpy as np
import ml_dtypes

import concourse.bass as bass
import concourse.tile as tile
from concourse import bacc
from concourse import mybir
from concourse.bass_utils import run_bass_kernel_spmd
from concourse.masks import make_identity

N, L, D = 8, 512, 256
H, QK, V = 16, 32, 32
OUT = 256
PI = 3.14159265358979323846

f32 = mybir.dt.float32
i16 = mybir.dt.int16
bf16 = mybir.dt.bfloat16
FP8 = mybir.dt.float8e4

AF = mybir.ActivationFunctionType
ALU = mybir.AluOpType
AX = mybir.AxisListType

# fp16 for all non-fp8 matmul operands: 1 cycle/row on the PE, 2x DVE modes
# on 2-byte copies, and enough mantissa for the sign-discontinuous spatial
# features (atan2 / dir).
MMDT = mybir.dt.float16

# fast-exp (Schraudolph in fp16 bitspace): i16(S*EXPA + EXPB) bitcast to fp16
# equals SCL * exp(S) * (1 +- 3.9% ripple). SCL cancels in softmax; ACT-exp
# tiles are biased by ln(SCL) so all tiles share one scale.
EXPA = 1477.3197  # 2^10 / ln 2
EXPB = 17348.8    # 15*2^10 (fp16 exp bias) + 2*2^10 (headroom) - 59.2
LNSC = 1.3854800628848514  # ln(measured SCL) under truncating f32->i16
WS = 16.0  # host weight pre-scale (fp8 subnormal avoidance), undone on-chip
RS = 8.0   # r1 (spatial-encoding hidden) pre-scale


def build_program():
    KGROUPS = int(os.environ.get("KGROUPS", "4"))
    # exp engine schedule: one char per [128,1024] exp tile in emission order
    # (8 early tiles then 24 pipelined), A=ACT true exp, D=DVE, P=Pool
    # PSUM can only be read by ACT ("A") and DVE ("D") -- the BIR verifier
    # rejects GPSIMD-PSUM access -- so every engine string below is A/D only.
    EXPSCHED = os.environ.get("EXPSCHED", "A" * 32)
    QCOPY = os.environ.get("QCOPY", "ADDD")  # qT PSUM->SBUF copy engine by mc
    KCOPY = os.environ.get("KCOPY", "DDDD")
    VENG = os.environ.get("VENG", "DDDD")    # Vaug value-mask engine by lc
    HRELU = os.environ.get("HRELU", "DADA")  # h1 relu engine by mc
    nc = bacc.Bacc()

    def inp(name, shape, dt=f32):
        return nc.declare_dram_parameter(name, list(shape), dt, isOutput=False)

    # packed DRAM inputs: few large DMAs instead of ~48 small ones
    pkQ_d = inp("pkQ", (128, 2048), MMDT)   # xT (2x512) | Wq (2x512)
    pkK_d = inp("pkK", (128, 1024), MMDT)   # Wk
    pkX8_d = inp("pkX8", (128, 1024), FP8)  # xT in fp8 (v DoubleRow lhsT)
    pkV_d = inp("pkV", (128, 1024), FP8)    # WvDR = [Wv0 | Wv1], x WS
    pkW_d = inp("pkW", (128, 3584), MMDT)   # otW1 | otW2
    pkM_d = inp("pkM", (128, 80))           # maskpm | mask16pm | posCB |
                                            # posCA | frame9 | seb2pm |
                                            # seb2x16pm | otb1pm
    pkC_d = inp("pkC", (128, 1792))         # x | otb2B | lngB | lnbB
    pkD_d = inp("pkD", (3, 544), MMDT)      # posCAT | seW1
    seW2_d = inp("seW2", (QK, H * QK), MMDT)
    seb1_d = inp("seb1", (QK, 1))
    out_d = nc.declare_dram_parameter("out", [L, OUT], f32, isOutput=True)

    DR = mybir.MatmulPerfMode.DoubleRow

    with tile.TileContext(nc) as tc, ExitStack() as ctx:
        consts = ctx.enter_context(tc.tile_pool(name="consts", bufs=1))
        wpool = ctx.enter_context(tc.tile_pool(name="weights", bufs=1))
        work = ctx.enter_context(tc.tile_pool(name="work", bufs=1))

        # ---- constants -------------------------------------------------
        ident = consts.tile([128, 128], MMDT)
        make_identity(nc, ident)
        # dummy exp hoists the single ACT table load off the critical path
        dm1 = consts.tile([1, 1], f32)
        nc.vector.memset(dm1, 0.0)
        nc.scalar.activation(out=dm1, in_=dm1, func=AF.Exp)
        e20 = consts.tile([128, 1], f32)
        nc.vector.memset(e20, 1e-20)
        e5 = consts.tile([128, 1], f32)
        nc.vector.memset(e5, 1e-5)
        lnsc = consts.tile([128, 1], f32)
        nc.vector.memset(lnsc, LNSC)

        # ---- input DMAs ------------------------------------------------
        def dma(t, src, eng=None):
            if not isinstance(src, bass.AP):
                src = src[:, :]
            (eng or nc.sync).dma_start(out=t, in_=src)

        pkD = wpool.tile([3, 544], MMDT, name="pkD")
        dma(pkD, pkD_d)
        pkQ = wpool.tile([128, 2048], MMDT, name="pkQ")
        dma(pkQ[:, 0:1024], pkQ_d[:, 0:1024])    # xT (gates everything)
        dma(pkQ[:, 1024:2048], pkQ_d[:, 1024:2048])  # Wq
        pkK = wpool.tile([128, 1024], MMDT, name="pkK")
        dma(pkK, pkK_d)
        seW2s = wpool.tile([32, 512], MMDT)
        dma(seW2s, seW2_d[:, :], eng=nc.gpsimd)
        seb1s = wpool.tile([32, 1], f32)
        dma(seb1s, seb1_d[:, :], eng=nc.gpsimd)
        pkM = wpool.tile([128, 80], f32, name="pkM")
        dma(pkM, pkM_d, eng=nc.gpsimd)
        pkX8 = wpool.tile([128, 1024], FP8, name="pkX8")
        dma(pkX8, pkX8_d)
        pkV = wpool.tile([128, 1024], FP8, name="pkV")
        dma(pkV, pkV_d)
        pkC = wpool.tile([128, 1792], f32, name="pkC")
        dma(pkC, pkC_d)
        pkW = wpool.tile([128, 3584], MMDT, name="pkW")
        dma(pkW, pkW_d)  # otW1 | otW2 (needed last)

        xTs = [pkQ[:, 512 * i : 512 * (i + 1)] for i in range(2)]
        Wqs = [pkQ[:, 1024 + 512 * i : 1024 + 512 * (i + 1)] for i in range(2)]
        Wks = [pkK[:, 512 * i : 512 * (i + 1)] for i in range(2)]
        x8dr = pkX8[:, :].rearrange("p (two n) -> p two n", two=2)
        wvdr = pkV[:, :].rearrange("p (two n) -> p two n", two=2)
        otW1s = [pkW[:, 512 * i : 512 * (i + 1)] for i in range(5)]
        otW2s = [pkW[:, 2560 + 256 * i : 2560 + 256 * (i + 1)] for i in range(4)]
        posCATs = pkD[:, 0:512]
        seW1s = pkD[:, 512:544]
        xn = [pkC[:, 256 * c : 256 * (c + 1)] for c in range(4)]
        otb2B = pkC[:, 1024:1280]
        lngB = pkC[:, 1280:1536]
        lnbB = pkC[:, 1536:1792]
        maskpm = pkM[:, 0:4]
        mask16pm = pkM[:, 4:8]
        posCBn = [pkM[:, 8 + 3 * c : 8 + 3 * (c + 1)] for c in range(4)]
        frame_sec = 32  # frame9 cols at 32 + 9c .. 68
        seb2s = pkM[:, 68:72]
        seb2x16 = pkM[:, 72:76]
        otb1s = pkM[:, 76:80]
        maskr = wpool.tile([128, 4], MMDT)
        nc.vector.tensor_copy(out=maskr, in_=maskpm)
        # frame columns broadcast over heads, hoisted off the spatial-phase
        # critical chain (Pool is idle during projections)
        fbA = wpool.tile([128, 576], f32, name="fbA")
        fbv = fbA.rearrange("p (k c h) -> p k c h", c=4, h=16)
        for k in range(9):
            nc.gpsimd.tensor_copy(
                out=fbv[:, k],
                in_=pkM[:, frame_sec + k : frame_sec + 36 : 9].unsqueeze(2)
                .broadcast_to([128, 4, 16]),
            )

        def _veng(c):
            assert c == "D", f"PSUM reader must be A or D, got {c}"
            return nc.vector

        exp_idx = [0]

        def make_E(ps_S, pool, name, tag=None, width=512):
            c = EXPSCHED[exp_idx[0] % len(EXPSCHED)]
            exp_idx[0] += 1
            E = pool.tile([128, width], MMDT, tag=tag, name=name)
            if c == "A":
                nc.scalar.activation(out=E, in_=ps_S, func=AF.Exp,
                                     bias=lnsc, scale=1.0)
            else:
                _veng(c).tensor_scalar(
                    out=E.bitcast(i16), in0=ps_S, scalar1=EXPA, scalar2=EXPB,
                    op0=ALU.mult, op1=ALU.add,
                )
            return E

        # ---- projections ----------------------------------------------
        qT, kT, Vaug = [], [], []
        with tc.tile_pool(name="psproj", bufs=1, space="PSUM") as psproj:
            # spatial encoding: r1 = relu(se_W1^T @ posCA^T + b1)
            ps_r1 = psproj.tile([128, 512], f32, tag="ps_v0", name="ps_r1")
            nc.tensor.matmul(out=ps_r1[0:32, :], lhsT=seW1s, rhs=posCATs,
                             start=True, stop=True)
            r1 = work.tile([32, 512], MMDT, name="r1")
            nc.scalar.activation(out=r1, in_=ps_r1[0:32, :], func=AF.Relu,
                                 bias=seb1s, scale=1.0)

            def emit_qk(mc):
                # k first: it does not wait on the r1/spat chain, so kT[mc]
                # lands early and the first S matmuls are q-gated only
                ps_k = psproj.tile([128, 512], f32, tag="ps_k")
                for kc in range(2):
                    nc.tensor.matmul(
                        out=ps_k,
                        lhsT=Wks[kc][:, 128 * mc : 128 * (mc + 1)],
                        rhs=xTs[kc], start=(kc == 0), stop=(kc == 1),
                    )
                t = work.tile([128, 512], MMDT, name=f"kT{mc}")
                c = KCOPY[mc]
                if c == "A":
                    nc.scalar.activation(out=t, in_=ps_k, func=AF.Copy)
                else:
                    nc.vector.tensor_copy(out=t, in_=ps_k)
                kT.append(t)

                ps_q = psproj.tile([128, 512], f32, tag="ps_q")
                for kc in range(2):
                    nc.tensor.matmul(
                        out=ps_q,
                        lhsT=Wqs[kc][:, 128 * mc : 128 * (mc + 1)],
                        rhs=xTs[kc], start=(kc == 0), stop=False,
                    )
                nc.tensor.matmul(
                    out=ps_q, lhsT=seW2s[:, 128 * mc : 128 * (mc + 1)],
                    rhs=r1, start=False, stop=True,
                )
                t = work.tile([128, 512], MMDT, name=f"qT{mc}")
                c = QCOPY[mc]
                if c == "A":
                    nc.scalar.activation(
                        out=t, in_=ps_q, func=AF.Identity,
                        bias=seb2s[:, mc : mc + 1], scale=1.0,
                    )
                else:
                    _veng(c).tensor_scalar(
                        out=t, in0=ps_q, scalar1=seb2s[:, mc : mc + 1],
                        scalar2=None, op0=ALU.add,
                    )
                qT.append(t)

            emit_qk(0)
            # heads 0-3 S+exp emitted mid-projections (they only need the
            # mc=0 chunk of qT/kT): the exp streams start ~6us earlier
            earlyEs = {}

            def emit_early(eh):
                ch, r = eh // 4, (eh % 4) * 32
                Es = []
                for half in range(2):
                    ps_S = psproj.tile(
                        [128, 1024], f32, tag=f"ps_S{half}",
                        name=f"psSe{eh}_{half}",
                    )
                    for kcl in range(2):
                        kc = 2 * half + kcl
                        nc.tensor.matmul(
                            out=ps_S[:, 512 * kcl : 512 * (kcl + 1)],
                            lhsT=kT[ch][r : r + 32,
                                        128 * kc : 128 * (kc + 1)],
                            rhs=qT[ch][r : r + 32, :],
                            start=True,
                            stop=True,
                            tile_position=(r, 0),
                        )
                    Es.append(make_E(ps_S, work, f"Ee{eh}_{half}", width=1024))
                earlyEs[eh] = Es

            emit_early(0)
            emit_qk(1)
            emit_early(1)

            for lc in range(4):
                ps_v = psproj.tile([128, 512], f32, tag=f"ps_v{lc % 2}")
                nc.tensor.matmul(
                    out=ps_v, lhsT=x8dr[:, :, 128 * lc : 128 * (lc + 1)],
                    rhs=wvdr, start=True, stop=True, perf_mode=DR,
                )
                va = work.tile([128, H * 36], MMDT, name=f"Vaug{lc}")
                vav = va.rearrange("p (h j) -> p h j", j=36)
                mcol = maskpm[:, lc : lc + 1]
                m16col = mask16pm[:, lc : lc + 1]
                c = VENG[lc]
                # value columns: (WS*v) * mask/WS  (kills the -INF logit
                # bias: masked keys contribute 0 to numerator and denom)
                if c == "A":
                    nc.scalar.activation(
                        out=vav[:, :, 0:32],
                        in_=ps_v.rearrange("p (h j) -> p h j", j=32),
                        func=AF.Copy, scale=m16col,
                    )
                else:
                    _veng(c).tensor_scalar(
                        out=vav[:, :, 0:32],
                        in0=ps_v.rearrange("p (h j) -> p h j", j=32),
                        scalar1=m16col, scalar2=None, op0=ALU.mult,
                    )
                # pos_CB columns (masked), replicated across heads
                pcbm = work.tile([128, 3], MMDT, tag="pcbm")
                nc.gpsimd.tensor_scalar(
                    out=pcbm, in0=posCBn[lc], scalar1=mcol, scalar2=None,
                    op0=ALU.mult,
                )
                nc.gpsimd.tensor_copy(
                    out=vav[:, :, 32:35],
                    in_=pcbm.unsqueeze(1).broadcast_to([128, H, 3]),
                )
                # ones column -> softmax denominator (masked)
                nc.gpsimd.tensor_copy(
                    out=vav[:, :, 35:36],
                    in_=maskr[:, lc : lc + 1].unsqueeze(1)
                    .broadcast_to([128, H, 1]),
                )
                Vaug.append(va)
                if lc == 0:
                    emit_early(2)
                elif lc == 1:
                    emit_early(3)
            emit_qk(2)
            emit_qk(3)

        # ---- attention -------------------------------------------------
        # Per head h: S^T = k_h q_h^T in [key, query] chunks, E ~ exp(S^T)
        # in fp16, then FLIPPED value matmuls: E chunks stationary, Vaug36
        # columns streaming so feat_node, apb and the softmax denominator
        # land in [query, feat] layout; normalization is per-partition.
        featT = [work.tile([128, 512], MMDT, name=f"featT{g}") for g in range(5)]
        featA = work.tile([128, 2048], MMDT, name="featA")
        apbA = work.tile([128, 256], f32, name="apbA")
        recALL = work.tile([128, 64], f32, name="recALL")  # [l, 16c+4g+s]

        with (
            tc.tile_pool(name="psS", bufs=int(os.environ.get("PSSBUFS", "3")), space="PSUM") as psS_pool,
            tc.tile_pool(name="psFU", bufs=1, space="PSUM") as psFU_pool,
            tc.tile_pool(name="Epool", bufs=6) as E_pool,
            tc.tile_pool(name="gwork", bufs=4) as gwork,
        ):
            psFUgs = {}
            psFUtiles = {}

            def emit_S(h):
                ch, r = h // 4, (h % 4) * 32
                Es = []
                for half in range(2):
                    ps_S = psS_pool.tile([128, 1024], f32, tag="ps_S")
                    for kcl in range(2):
                        kc = 2 * half + kcl
                        nc.tensor.matmul(
                            out=ps_S[:, 512 * kcl : 512 * (kcl + 1)],
                            lhsT=kT[ch][r : r + 32, 128 * kc : 128 * (kc + 1)],
                            rhs=qT[ch][r : r + 32, :],
                            start=True,
                            stop=True,
                            tile_position=(r, 0),
                        )
                    Es.append(make_E(ps_S, E_pool, f"E{h}_{half}", tag="E",
                                     width=1024))
                return Es

            def emit_AV(h, Es):
                g, s = h // 4, h % 4
                if s == 0:
                    psFUa = psFU_pool.tile([128, 288], f32, tag="psFUa",
                                           name=f"psFUa_{g}")
                    psFUb = psFU_pool.tile([128, 288], f32, tag="psFUb",
                                           name=f"psFUb_{g}")
                    psFUtiles[g] = (psFUa, psFUb)
                    psFUgs[g] = [
                        (psFUa, psFUb)[c // 2][
                            :, 144 * (c % 2) : 144 * (c % 2 + 1)]
                        for c in range(4)
                    ]
                psFUg = psFUgs[g]
                for c in range(4):
                    for kc in range(4):
                        nc.tensor.matmul(
                            out=psFUg[c][:, 36 * s : 36 * (s + 1)],
                            lhsT=Es[kc // 2][
                                :,
                                512 * (kc % 2) + 128 * c :
                                512 * (kc % 2) + 128 * (c + 1),
                            ],
                            rhs=Vaug[kc][:, 36 * h : 36 * (h + 1)],
                            start=(kc == 0),
                            stop=(kc == 3),
                        )

            A4 = apbA.rearrange("p (c h j) -> p c h j", h=16, j=4)
            RC = recALL.rearrange("p (c h) -> p c h", h=16)
            FA = featA.rearrange("p (c g2 s j) -> p c g2 s j", g2=4, s=4, j=32)

            # spatial stage-1 (no ACT): apb normalize + frame-rotated pts +
            # square-sums, emitted per head-half as soon as that half's
            # groups finish -- DVE/Pool are idle during the ACT exp stream
            recB4 = work.tile([128, 256], f32, name="recB4")
            rb4 = recB4.rearrange("p (c h j) -> p c h j", h=16, j=4)
            spatA = work.tile([128, 512], MMDT, name="spatA")
            spatc = spatA.rearrange("p (c f) -> p c f", f=128)
            ptsF = work.tile([128, 192], f32, name="ptsF")
            ptsv = ptsF.rearrange("p (c h i) -> p c h i", h=16, i=3)
            sqA = work.tile([128, 192], f32, name="sqA")
            sqav = sqA.rearrange("p (c h i) -> p c h i", h=16, i=3)
            sqB = work.tile([128, 192], f32, name="sqB")
            sqbv = sqB.rearrange("p (c h i) -> p c h i", h=16, i=3)
            tDistA = work.tile([128, 64], f32, name="tDistA")
            tDistv = tDistA.rearrange("p (c h) -> p c h", h=16)
            tDirA = work.tile([128, 64], f32, name="tDirA")
            tDirv = tDirA.rearrange("p (c h) -> p c h", h=16)
            spT1 = work.tile([128, 64], f32, name="spT1")
            spT1v = spT1.rearrange("p (c h) -> p c h", h=16)
            spT2 = work.tile([128, 64], f32, name="spT2")
            spT2v = spT2.rearrange("p (c h) -> p c h", h=16)
            spT3 = work.tile([128, 64], f32, name="spT3")
            spT3v = spT3.rearrange("p (c h) -> p c h", h=16)
            spT4 = work.tile([128, 64], f32, name="spT4")
            spT4v = spT4.rearrange("p (c h) -> p c h", h=16)

            def spatial_stage1(h0, h1):
                n = h1 - h0
                hs = slice(h0, h1)
                A4h = A4[:, :, hs, :]
                rbh = rb4[:, :, hs, :]
                nc.vector.tensor_copy(
                    out=rbh,
                    in_=RC[:, :, hs].unsqueeze(3).broadcast_to([128, 4, n, 4]),
                )
                nc.vector.tensor_tensor(out=A4h, in0=A4h, in1=rbh, op=ALU.mult)
                nc.vector.tensor_tensor(
                    out=A4h[:, :, :, 0:3], in0=A4h[:, :, :, 0:3],
                    in1=pkM[:, 20:32].rearrange("p (c j) -> p c j", j=3)
                    .unsqueeze(2).broadcast_to([128, 4, n, 3]),
                    op=ALU.subtract,
                )
                for i in range(3):
                    eng = nc.vector if i == 1 else nc.gpsimd
                    ta, tb = (spT1v, spT2v) if i == 1 else (spT3v, spT4v)
                    ta, tb = ta[:, :, hs], tb[:, :, hs]
                    eng.tensor_tensor(
                        out=ta, in0=A4h[:, :, :, 0],
                        in1=fbv[:, 3 * i + 0][:, :, hs], op=ALU.mult,
                    )
                    eng.tensor_tensor(
                        out=tb, in0=A4h[:, :, :, 1],
                        in1=fbv[:, 3 * i + 1][:, :, hs], op=ALU.mult,
                    )
                    eng.tensor_tensor(out=ta, in0=ta, in1=tb, op=ALU.add)
                    eng.tensor_tensor(
                        out=tb, in0=A4h[:, :, :, 2],
                        in1=fbv[:, 3 * i + 2][:, :, hs], op=ALU.mult,
                    )
                    eng.tensor_tensor(
                        out=ptsv[:, :, hs, i], in0=ta, in1=tb, op=ALU.add
                    )
                nc.vector.tensor_copy(
                    out=spatc[:, :, 3 * h0 : 3 * h1],
                    in_=ptsF.rearrange("p (c f) -> p c f", f=48)
                    [:, :, 3 * h0 : 3 * h1],
                )
                nc.gpsimd.tensor_tensor(
                    out=sqav[:, :, hs, :], in0=A4h[:, :, :, 0:3],
                    in1=A4h[:, :, :, 0:3], op=ALU.mult,
                )
                nc.vector.tensor_reduce(
                    out=tDistv[:, :, hs], in_=sqav[:, :, hs, :],
                    axis=AX.X, op=ALU.add,
                )
                nc.gpsimd.tensor_tensor(
                    out=sqbv[:, :, hs, :], in0=ptsv[:, :, hs, :],
                    in1=ptsv[:, :, hs, :], op=ALU.mult,
                )
                nc.vector.tensor_reduce(
                    out=tDirv[:, :, hs], in_=sqbv[:, :, hs, :],
                    axis=AX.X, op=ALU.add,
                )

            def emit_post(g):
                # apb + denom rows out of PSUM (2 merged copies), merged
                # per-query reciprocal, one Pool broadcast, then the
                # normalization fused into the PSUM->SBUF feat copy
                psFUa, psFUb = psFUtiles[g]
                for ca, pst in enumerate((psFUa, psFUb)):
                    nc.vector.tensor_copy(
                        out=A4[:, 2 * ca : 2 * ca + 2, 4 * g : 4 * g + 4, :],
                        in_=pst.rearrange("p (c s j) -> p c s j", s=4, j=36)
                        [:, :, :, 32:36],
                    )
                nc.vector.reciprocal(
                    out=RC[:, :, 4 * g : 4 * g + 4],
                    in_=A4[:, :, 4 * g : 4 * g + 4, 3],
                )
                bcg = gwork.tile([128, 512], f32, tag="bcg")
                bcv = bcg.rearrange("p (c s j) -> p c s j", s=4, j=32)
                nc.vector.tensor_copy(
                    out=bcv,
                    in_=RC[:, :, 4 * g : 4 * g + 4].unsqueeze(3)
                    .broadcast_to([128, 4, 4, 32]),
                )
                for ca, pst in enumerate((psFUa, psFUb)):
                    nc.vector.tensor_tensor(
                        out=FA[:, 2 * ca : 2 * ca + 2, g, :, :],
                        in0=pst.rearrange("p (c s j) -> p c s j", s=4, j=36)
                        [:, :, :, 0:32],
                        in1=bcv[:, 2 * ca : 2 * ca + 2, :, :],
                        op=ALU.mult,
                    )

            def emit_post_tr(g):
                # feat_node transpose for group g: [q, feat] -> [feat, q];
                # output borrows a rotating ps_S slot (bitcast to fp16) so no
                # dedicated PSUM bank is needed
                psFT_raw = psS_pool.tile([128, 1024], f32, tag="ps_S",
                                         name=f"psFT{g}")
                psFT = psFT_raw.bitcast(MMDT)[:, 0:512]
                for c in range(4):
                    nc.tensor.transpose(
                        out=psFT[:, 128 * c : 128 * (c + 1)],
                        in_=featA[:, 512 * c + 128 * g : 512 * c + 128 * g + 128],
                        identity=ident,
                    )
                nc.vector.tensor_copy(out=featT[g], in_=psFT)

            # software-pipelined: S(h+1) is emitted BEFORE AV(h) so the
            # in-order PE stream computes the next head's logits during
            # exp(h) instead of stalling behind AV matmuls waiting on E(h)
            KLAG = int(os.environ.get("KLAG", "4"))
            pend = []

            def drain_one():
                hp, Esp = pend.pop(0)
                emit_AV(hp, Esp)
                if hp % 4 == 3:
                    emit_post(hp // 4)
                    if hp // 4 == 1:
                        spatial_stage1(0, 8)
                    elif hp // 4 == 2:
                        spatial_stage1(8, 12)
                if hp % 4 == 2 and hp // 4 > 0:
                    emit_post_tr(hp // 4 - 1)

            for h in range(4 * KGROUPS):
                Es = earlyEs.pop(h) if h in earlyEs else emit_S(h)
                pend.append((h, Es))
                if len(pend) > KLAG:
                    drain_one()
            while pend:
                drain_one()
            emit_post_tr(KGROUPS - 2)
            emit_post_tr(KGROUPS - 1)
            spatial_stage1(12, 16)

        with tc.tile_pool(name="spwork", bufs=1) as spw:
            tA = spw.tile([128, 64], f32, name="tA")
            tB = spw.tile([128, 64], f32, name="tB")
            tAv = tA.rearrange("p (c h) -> p c h", h=16)
            tBv = tB.rearrange("p (c h) -> p c h", h=16)
            axM = spw.tile([128, 64], f32, name="axM")
            ayM = spw.tile([128, 64], f32, name="ayM")
            qsM = spw.tile([128, 64], f32, name="qsM")
            axv = axM.rearrange("p (c h) -> p c h", h=16)
            ayv = ayM.rearrange("p (c h) -> p c h", h=16)
            qsv = qsM.rearrange("p (c h) -> p c h", h=16)
            a2 = spw.tile([128, 64], f32, name="a2")
            a2v = a2.rearrange("p (c h) -> p c h", h=16)
            a4 = spw.tile([128, 64], f32, name="a4")
            a4v = a4.rearrange("p (c h) -> p c h", h=16)
            pA = spw.tile([128, 64], f32, name="pA")
            pAv = pA.rearrange("p (c h) -> p c h", h=16)
            pB = spw.tile([128, 64], f32, name="pB")
            pBv = pB.rearrange("p (c h) -> p c h", h=16)
            usM = spw.tile([128, 64], f32, name="usM")
            usv = usM.rearrange("p (c h) -> p c h", h=16)
            wsM = spw.tile([128, 64], f32, name="wsM")
            wsv = wsM.rearrange("p (c h) -> p c h", h=16)
            sM = spw.tile([128, 64], f32, name="sM")
            sv = sM.rearrange("p (c h) -> p c h", h=16)
            u2M = spw.tile([128, 64], f32, name="u2M")
            u2v = u2M.rearrange("p (c h) -> p c h", h=16)

            def spatial_stage2(h0, h1):
                # sqrt/dir/atan for one head-half; half 0-7 runs while half
                # 8-15's stage-1 is still on DVE/Pool
                n = h1 - h0
                hs = slice(h0, h1)
                # dist (first Sqrt after the last exp -> one table switch)
                nc.scalar.activation(
                    out=spatc[:, :, 48 + h0 : 48 + h1], in_=tDistv[:, :, hs],
                    func=AF.Sqrt, bias=e20,
                )
                # dir = pts / sqrt(p2 + tiny)
                nc.scalar.activation(out=tBv[:, :, hs], in_=tDirv[:, :, hs],
                                     func=AF.Sqrt, bias=e20)
                nc.vector.reciprocal(out=tBv[:, :, hs], in_=tBv[:, :, hs])
                nc.gpsimd.tensor_tensor(
                    out=spatc[:, :, 64 + 3 * h0 : 64 + 3 * h1]
                    .rearrange("p c (h i) -> p c h i", i=3),
                    in0=ptsv[:, :, hs, :],
                    in1=tBv[:, :, hs].unsqueeze(3)
                    .broadcast_to([128, 4, n, 3]),
                    op=ALU.mult,
                )
                # atan2 range reduction: a = min(|x|,|y|) / max(|x|,|y|)
                pxh, pyh = ptsv[:, :, hs, 0], ptsv[:, :, hs, 1]
                axh, ayh = axv[:, :, hs], ayv[:, :, hs]
                qsh = qsv[:, :, hs]
                tAh, tBh = tAv[:, :, hs], tBv[:, :, hs]
                nc.scalar.activation(out=axh, in_=pxh, func=AF.Abs)
                nc.scalar.activation(out=ayh, in_=pyh, func=AF.Abs)
                nc.vector.tensor_tensor(out=tAh, in0=axh, in1=ayh, op=ALU.min)
                nc.vector.scalar_tensor_tensor(
                    out=tBh, in0=axh, scalar=1e-38, in1=ayh,
                    op0=ALU.add, op1=ALU.max,
                )
                nc.vector.reciprocal(out=tBh, in_=tBh)
                nc.gpsimd.tensor_tensor(out=qsh, in0=tAh, in1=tBh, op=ALU.mult)
                ang = spatc[:, :, 112 + h0 : 112 + h1]
                # masks / sign on DVE (parallel with the Pool poly chain):
                #   atan2 = P(q)*q*us + ws,  us = (1-2m1)(1-2m2)*s,
                #   ws = (m1*(pi/2)*(1-2m2) + m2*pi)*s
                svh = sv[:, :, hs]
                u2h = u2v[:, :, hs]
                ush = usv[:, :, hs]
                wsh = wsv[:, :, hs]
                nc.vector.tensor_tensor(out=tAh, in0=ayh, in1=axh, op=ALU.is_gt)
                nc.vector.tensor_scalar(
                    out=tBh, in0=pxh, scalar1=0.0, scalar2=None, op0=ALU.is_lt
                )
                nc.vector.tensor_scalar(
                    out=svh, in0=pyh, scalar1=0.0, scalar2=2.0,
                    op0=ALU.is_ge, op1=ALU.mult,
                )
                nc.vector.tensor_scalar(
                    out=svh, in0=svh, scalar1=-1.0, scalar2=None, op0=ALU.add
                )
                nc.vector.tensor_scalar(
                    out=u2h, in0=tBh, scalar1=-2.0, scalar2=1.0,
                    op0=ALU.mult, op1=ALU.add,
                )
                nc.vector.tensor_scalar(
                    out=ush, in0=tAh, scalar1=-2.0, scalar2=1.0,
                    op0=ALU.mult, op1=ALU.add,
                )
                nc.vector.tensor_tensor(out=ush, in0=ush, in1=u2h, op=ALU.mult)
                nc.vector.scalar_tensor_tensor(
                    out=wsh, in0=tAh, scalar=PI / 2, in1=u2h,
                    op0=ALU.mult, op1=ALU.mult,
                )
                nc.vector.scalar_tensor_tensor(
                    out=wsh, in0=tBh, scalar=PI, in1=wsh,
                    op0=ALU.mult, op1=ALU.add,
                )
                nc.vector.tensor_tensor(out=ush, in0=ush, in1=svh, op=ALU.mult)
                nc.vector.tensor_tensor(out=wsh, in0=wsh, in1=svh, op=ALU.mult)
                # Estrin poly on Pool
                a2h, a4h = a2v[:, :, hs], a4v[:, :, hs]
                pAh, pBh = pAv[:, :, hs], pBv[:, :, hs]
                nc.gpsimd.tensor_tensor(out=a2h, in0=qsh, in1=qsh, op=ALU.mult)
                nc.gpsimd.tensor_scalar(
                    out=pAh, in0=a2h, scalar1=-0.0851330, scalar2=0.1801410,
                    op0=ALU.mult, op1=ALU.add,
                )
                nc.gpsimd.tensor_scalar(
                    out=pBh, in0=a2h, scalar1=-0.3302995, scalar2=0.9998660,
                    op0=ALU.mult, op1=ALU.add,
                )
                nc.gpsimd.tensor_tensor(out=a4h, in0=a2h, in1=a2h, op=ALU.mult)
                nc.gpsimd.tensor_tensor(out=pAh, in0=pAh, in1=a4h, op=ALU.mult)
                nc.gpsimd.tensor_tensor(out=pAh, in0=pAh, in1=pBh, op=ALU.add)
                nc.gpsimd.tensor_tensor(out=pAh, in0=pAh, in1=qsh, op=ALU.mult)
                nc.vector.tensor_tensor(out=pAh, in0=pAh, in1=ush, op=ALU.mult)
                nc.vector.tensor_tensor(out=ang, in0=pAh, in1=wsh, op=ALU.add)

            spatial_stage2(0, 8)
            spatial_stage2(8, 16)

        # ---- MLP part 1: h1 partial accumulation over feat_node ---------
        xb = []
        for c in range(4):
            t = work.tile([128, 256], f32, name=f"xb{c}")
            nc.gpsimd.tensor_scalar(
                out=t, in0=otb2B, scalar1=maskpm[:, c : c + 1], scalar2=None,
                op0=ALU.mult,
            )
            nc.gpsimd.tensor_tensor(out=t, in0=t, in1=xn[c], op=ALU.add)
            xb.append(t)
        h1T = [work.tile([128, 512], MMDT, name=f"h1T{mc}") for mc in range(4)]
        psM_pool = ctx.enter_context(tc.tile_pool(name="psM", bufs=4, space="PSUM"))
        ps_hs = []
        for mc in range(4):
            ps_h = psM_pool.tile([128, 512], f32, tag="ps_h", name=f"ps_h{mc}")
            ps_hs.append(ps_h)
            for kc in range(4):
                nc.tensor.matmul(
                    out=ps_h,
                    lhsT=otW1s[kc][:, 128 * mc : 128 * (mc + 1)],
                    rhs=featT[kc],
                    start=(kc == 0),
                    stop=False,
                )
        # ---- spatial features, post-attention remainder (ACT sqrt parts
        # and atan2; normalize/pts/square-sums ran in spatial_stage1) ------

        # transpose spatial features into featT[4]
        with tc.tile_pool(name="psSp", bufs=1, space="PSUM") as psSp_pool:
            ps_sp = psSp_pool.tile([128, 512], MMDT)
            for c in range(4):
                nc.tensor.transpose(
                    out=ps_sp[:, 128 * c : 128 * (c + 1)],
                    in_=spatA[:, 128 * c : 128 * (c + 1)],
                    identity=ident,
                )
            nc.vector.tensor_copy(out=featT[4], in_=ps_sp)

        # ---- MLP part 2: kc=4 finals, relu, out-stage, layernorm --------
        with tc.tile_pool(name="psO", bufs=1, space="PSUM") as psO_pool:
            for mc in range(4):
                nc.tensor.matmul(
                    out=ps_hs[mc],
                    lhsT=otW1s[4][:, 128 * mc : 128 * (mc + 1)],
                    rhs=featT[4],
                    start=False,
                    stop=True,
                )
                if HRELU[mc] == "A":
                    nc.scalar.activation(
                        out=h1T[mc], in_=ps_hs[mc], func=AF.Relu,
                        bias=otb1s[:, mc : mc + 1], scale=1.0,
                    )
                else:
                    nc.vector.tensor_scalar(
                        out=h1T[mc], in0=ps_hs[mc],
                        scalar1=otb1s[:, mc : mc + 1], scalar2=0.0,
                        op0=ALU.add, op1=ALU.max,
                    )
            ps_os = [
                psO_pool.tile([128, 256], f32, tag=f"ps_o{lc}", name=f"ps_o{lc}")
                for lc in range(4)
            ]
            for lc in range(4):
                for kc in range(4):
                    nc.tensor.matmul(
                        out=ps_os[lc],
                        lhsT=h1T[kc][:, 128 * lc : 128 * (lc + 1)],
                        rhs=otW2s[kc],
                        start=(kc == 0),
                        stop=(kc == 3),
                    )
            for lc in range(4):
                y = work.tile([128, 256], f32, name=f"y_ln{lc}")
                if lc % 2 == 0:
                    nc.vector.scalar_tensor_tensor(
                        out=y, in0=ps_os[lc], scalar=maskpm[:, lc : lc + 1],
                        in1=xb[lc], op0=ALU.mult, op1=ALU.add,
                    )
                else:
                    t = work.tile([128, 256], f32, name=f"yt{lc}")
                    nc.scalar.activation(
                        out=t, in_=ps_os[lc], func=AF.Copy,
                        scale=maskpm[:, lc : lc + 1],
                    )
                    nc.gpsimd.tensor_tensor(out=y, in0=t, in1=xb[lc],
                                            op=ALU.add)
                stats = work.tile([128, 6], f32, name=f"ln_stats{lc}")
                mv = work.tile([128, 2], f32, name=f"ln_mv{lc}")
                nc.vector.bn_stats(out=stats, in_=y)
                nc.vector.bn_aggr(out=mv, in_=stats)
                mu = mv[:, 0:1]
                rstd = work.tile([128, 1], f32, name=f"ln_rstd{lc}")
                nc.scalar.activation(out=rstd, in_=mv[:, 1:2], func=AF.Sqrt,
                                     bias=e5)
                nc.vector.reciprocal(out=rstd, in_=rstd)
                z = work.tile([128, 256], f32, name=f"z_out{lc}")
                if lc % 2 == 0:
                    nc.vector.scalar_tensor_tensor(
                        out=z, in0=y, scalar=mu, in1=lngB,
                        op0=ALU.subtract, op1=ALU.mult,
                    )
                    nc.vector.scalar_tensor_tensor(
                        out=z, in0=z, scalar=rstd, in1=lnbB,
                        op0=ALU.mult, op1=ALU.add,
                    )
                else:
                    nc.gpsimd.tensor_scalar(
                        out=z, in0=y, scalar1=mu, scalar2=rstd,
                        op0=ALU.subtract, op1=ALU.mult,
                    )
                    nc.gpsimd.tensor_tensor(out=z, in0=z, in1=lngB, op=ALU.mult)
                    nc.gpsimd.tensor_tensor(out=z, in0=z, in1=lnbB, op=ALU.add)
                nc.sync.dma_start(out=out_d[128 * lc : 128 * (lc + 1), :], in_=z)

    nc.finalize()
    return nc


_cached = {}


def _get_program():
    if "nc" not in _cached:
        _cached["nc"] = build_program()
    return _cached["nc"]


def _prep_core_inputs(x, pos_CA, pos_CB, frame, mask, Wq, Wk, Wv, seW1, seb1,
                      seW2, seb2, otW1, otb1, otW2, otb2, lng, lnb):
    f = np.float32
    h16 = np.float16
    f8 = ml_dtypes.float8_e4m3
    xT = x.T.astype(f)                                   # (256, 512)

    Wv16 = (Wv * WS).astype(f)
    pkQ = np.concatenate(
        [xT[0:128], xT[128:256], Wq[0:128], Wq[128:256]], axis=1).astype(f)
    pkK = np.concatenate([Wk[0:128], Wk[128:256]], axis=1).astype(f)
    pkX8 = np.concatenate([xT[0:128], xT[128:256]], axis=1)
    pkV = np.concatenate([Wv16[0:128], Wv16[128:256]], axis=1)
    pkW = np.concatenate(
        [otW1[128 * i : 128 * (i + 1)] for i in range(5)]
        + [otW2[128 * i : 128 * (i + 1)] for i in range(4)], axis=1).astype(f)
    maskf = mask.astype(f).reshape(4, 128).T
    pkM = np.concatenate(
        [maskf, maskf / WS]
        + [pos_CB.astype(f).reshape(4, 128, 3).transpose(1, 0, 2).reshape(128, 12)]
        + [pos_CA.astype(f).reshape(4, 128, 3).transpose(1, 0, 2).reshape(128, 12)]
        + [frame.astype(f).reshape(4, 128, 9).transpose(1, 0, 2).reshape(128, 36)]
        + [seb2.astype(f).reshape(4, 128).T,
           (seb2 * WS).astype(f).reshape(4, 128).T,
           otb1.astype(f).reshape(4, 128).T], axis=1).astype(f)
    xr = x.astype(f).reshape(4, 128, D)
    pkC = np.concatenate(
        [xr[c] for c in range(4)]
        + [np.tile(otb2.astype(f), (128, 1)),
           np.tile(lng.astype(f), (128, 1)),
           np.tile(lnb.astype(f), (128, 1))], axis=1).astype(f)
    pkD = np.concatenate([pos_CA.T.astype(f), seW1.astype(f)], axis=1)
    return {
        "pkQ": np.ascontiguousarray(pkQ.astype(h16)),
        "pkK": np.ascontiguousarray(pkK.astype(h16)),
        "pkX8": np.ascontiguousarray(pkX8.astype(f8)),
        "pkV": np.ascontiguousarray(pkV.astype(f8)),
        "pkW": np.ascontiguousarray(pkW.astype(h16)),
        "pkM": np.ascontiguousarray(pkM),
        "pkC": np.ascontiguousarray(pkC),
        "pkD": np.ascontiguousarray(pkD.astype(h16)),
        "seW2": np.ascontiguousarray(seW2.astype(h16)),
        "seb1": np.ascontiguousarray(seb1.reshape(QK, 1), dtype=f),
    }


def kernel(x, pos_CA, pos_CB, frame, mask,
           Wq, Wk, Wv, se_W1, se_b1, se_W2, se_b2,
           ot_W1, ot_b1, ot_W2, ot_b2, ln_g, ln_b):
    x = np.asarray(x, dtype=np.float32)
    pos_CA = np.asarray(pos_CA, dtype=np.float32)
    pos_CB = np.asarray(pos_CB, dtype=np.float32)
    frame = np.asarray(frame, dtype=np.float32)
    mask = np.asarray(mask)
    weights = [np.asarray(w, dtype=np.float32) for w in
               (Wq, Wk, Wv, se_W1, se_b1, se_W2, se_b2,
                ot_W1, ot_b1, ot_W2, ot_b2, ln_g, ln_b)]

    nc = _get_program()
    in_maps = [
        _prep_core_inputs(x[i], pos_CA[i], pos_CB[i], frame[i], mask[i], *weights)
        for i in range(N)
    ]
    res = run_bass_kernel_spmd(nc, in_maps, list(range(N)))
    _cached["last_results"] = res
    out = np.stack([res.results[i]["out"] for i in range(N)], axis=0)
    return out.astype(np.float32)
